# revision 7
# baseline (speedup 1.0000x reference)
"""Bahdanau-attention kernel for Trainium2, 8-core data-parallel over batch.

Problem: context = softmax(w2 . tanh(enc @ W1_enc + hid @ W1_hid + b1)) @ enc
  B=32, S=2048, D=1024.  Each of the 8 cores handles 4 batch elements.

Device-side strategy (per core, per batch b, per 512-wide seq chunk):
  - The big matmul runs in fp8e4 (e4m3) DoubleRow perf mode: both enc and
    W1_enc are e4m3, each matmul contracts K=256 (two 128-partition planes)
    at ~1 cycle/row-pair -- measured 2.15x the bf16 MAC rate on HW.
    h^T[m-chunk] = sum_kp Wpair[kp,m]^T @ encpair[kp]  (PE, PSUM f32 accum)
  - Quantization: enc_q = e4m3(enc x8) host-prescaled; W planes at x128.
    NPL selects accuracy/speed: 4 planes = W_hi only (K=1024); 6/8 add
    e4m3 residual (lo) planes for kp subsets, cutting the systematic W
    quantization error ~3%->0.3% on corrected ranges at +2 matmuls/(m,ss).
    Product scale 1024 is undone by the tanh's scale.
  - tanh+bias via ACT: h = tanh(hp/1024 + z), z[m] = (hid @ W1_hid + b1)[m]
    (z path keeps the old e3m4 x128 W1_hid weights; 64 tiny matmuls)
  - e-scores in row form: e_row[1, 512] = sum_m w2[m]^T @ h^T[m]  (PE,
    bf16 h moving), emitted one m-step behind the h matmuls
  - p_row = exp(e_row) on ACT with fused accum_out giving the softmax
    normalizer partial (no max subtraction: |e| <= sum|w2| ~ 26, safe fp32)
  - p broadcast to 128 partitions via K=1 ones-matmul on PE; exp/broadcast/
    context work for chunk i is emitted inside chunk i+1's matmul stream
  - ctx contribution on the DVE from a SEPARATE bf16 enc copy (encTb):
    the fp8 enc would put its 3% quantization error directly on the
    output; bf16 keeps the ctx path at the old accuracy. Both enc copies
    are host-laid-out [b, ss, p, k*512+s] so every per-chunk DMA is one
    fully contiguous 4KB(8KB)/partition block.
  - finalize once per invocation, off the critical path (see git history:
    1/Z via exp(-ln Z) on ACT, one contiguous [P, BL*KC] store).
"""

import numpy as np
from contextlib import ExitStack

import concourse.bacc as bacc
import concourse.tile as tile
from concourse import mybir
from concourse.bass_utils import run_bass_kernel_spmd

AFT = mybir.ActivationFunctionType
ALU = mybir.AluOpType
F32 = mybir.dt.float32

B, S, D = 32, 2048, 1024
NCORES = 8
BL = B // NCORES          # 4 batch elements per core
P = 128
KC = D // P               # 8 contraction / output chunks
S_SUB = 512               # seq chunk processed per inner iteration
NSS = S // S_SUB          # 4
NKP = KC // 2             # 4 DoubleRow k-pair planes in W_hi

# moving-side dtype (activations) and weight dtypes
DT = mybir.dt.bfloat16
WT = mybir.dt.float8e3    # e3m4 for the z-path weights (legacy)
WT8 = mybir.dt.float8e4   # e4m3: the only dtype DoubleRow accepts
W1E_SCALE = 128.0         # W1 prescale (planes hi+lo share it)
ENC_SCALE = 8.0           # enc prescale for e4m3
NPL = 4                   # DoubleRow planes: 4=hi only, 6/8=+lo residuals


def _body(ctx, tc, encT8, encTb, w1e8, hidT, w1h, b1r, onesb, w2, onesr, out):
    nc = tc.nc
    # bufs=2 on the weight/const pools: in the For_i timing loop the next
    # rep's weight DMA then overlaps this rep's tail compute instead of
    # serializing behind the last matmul that reads the old weights.
    const = ctx.enter_context(tc.tile_pool(name="const", bufs=2))
    wpool = ctx.enter_context(tc.tile_pool(name="wpool", bufs=2))
    epool8 = ctx.enter_context(tc.tile_pool(name="epool8", bufs=8))
    epoolb = ctx.enter_context(tc.tile_pool(name="epoolb", bufs=8))
    spool2 = ctx.enter_context(tc.tile_pool(name="spool2", bufs=2))
    # h tiles for a whole batch stay resident: the e-score matmuls run as
    # one contiguous bf16 block at batch end, so the PE pays one
    # DoubleRow<->bf16 mode-switch pair per batch instead of eight
    # (measured ~0.86us per switch pair on HW)
    hpool = ctx.enter_context(tc.tile_pool(name="hpool", bufs=36))
    spool = ctx.enter_context(tc.tile_pool(name="spool", bufs=2))
    cpool = ctx.enter_context(tc.tile_pool(name="cpool", bufs=2 * NSS + 2))
    fpool = ctx.enter_context(tc.tile_pool(name="fpool", bufs=2))
    pbpool = ctx.enter_context(tc.tile_pool(name="pbpool", bufs=2))
    # PSUM budget (8 banks): hp 6 + e-banks 2. Each e-bank hosts two
    # [2,512] e-score accumulators at partition offsets 0 and 32. The
    # p-broadcast lives on GpSimd now, so no PE/PSUM bank for it; the
    # sixth hp buf decouples the matmul stream from tanh-read latency.
    ppa = ctx.enter_context(tc.tile_pool(name="ppa", bufs=6, space="PSUM"))
    ppe = ctx.enter_context(tc.tile_pool(name="ppe", bufs=2, space="PSUM"))

    # --- phase 0: coalesced DMAs. z matmuls are first in the PE stream,
    # so w1h/hid go first; then the DoubleRow weight planes + first chunk.
    w1h_all = wpool.tile([P, KC * D], WT, name="w1h_all")
    nc.sync.dma_start(w1h_all[:].rearrange("p (k m) -> p k m", k=KC),
                      w1h.rearrange("(k p) m -> p k m", p=P))
    hid_all = const.tile([P, KC * BL], DT, name="hid_all")
    nc.sync.dma_start(hid_all[:].rearrange("p (k b) -> p k b", k=KC),
                      hidT.rearrange("(k p) b -> p k b", p=P))
    w1e_all = wpool.tile([P, KC * NPL * 2 * P], WT8, name="w1e_all")
    nc.sync.dma_start(w1e_all[:], w1e8[:, :])
    et8_0 = epool8.tile([P, KC * S_SUB], WT8, name="et8", tag="et8")
    nc.sync.dma_start(et8_0[:], encT8[0, 0])
    etb_0 = epoolb.tile([P, KC * S_SUB], DT, name="etb", tag="etb")
    nc.sync.dma_start(etb_0[:], encTb[0, 0])
    b1r_all = const.tile([1, D], DT, name="b1r_all")
    nc.sync.dma_start(b1r_all[:], b1r[:, :])
    onesb_t = const.tile([1, BL], DT, name="onesb_t")
    nc.sync.dma_start(onesb_t[:], onesb[:, :])
    w2_all = const.tile([P, KC * 2], DT, name="w2_all")
    nc.sync.dma_start(w2_all[:].rearrange("p (k c) -> p k c", k=KC),
                      w2.rearrange("(k p) c -> p k c", p=P))
    onesr_t = const.tile([1, P], WT, name="onesr_t")
    nc.sync.dma_start(onesr_t[:], onesr[:])
    w1h_t = [w1h_all[:, k * D:(k + 1) * D] for k in range(KC)]
    hid_t = [hid_all[:, k * BL:(k + 1) * BL] for k in range(KC)]
    w2_t = [w2_all[:, k * 2:(k + 1) * 2] for k in range(KC)]

    def w1e_pl(m, pl):
        blk = m * NPL + pl
        return w1e_all[:, blk * 2 * P:(blk + 1) * 2 * P].rearrange(
            "p (two q) -> p two q", two=2)

    # per-batch bias z = hid @ W1_hid + b1 (W1_hid is fp8 x128-prescaled;
    # the ACT copy undoes the scale and adds b1 in the same pass)
    z_sb = []
    for m in range(KC):
        zp = ppa.tile([P, BL], F32, name="zp", tag="ppa_t")
        for k in range(KC):
            nc.tensor.matmul(
                zp[:], lhsT=w1h_t[k][:, m * P:(m + 1) * P], rhs=hid_t[k],
                start=(k == 0), stop=False)
        # b1 folded in as a K=1 rank-1 matmul (host prescaled x128), so the
        # zt copy below needs no per-partition bias and can run on the ACT
        nc.tensor.matmul(zp[:], lhsT=b1r_all[:, m * P:(m + 1) * P],
                         rhs=onesb_t[:], start=False, stop=True)
        zt = const.tile([P, BL], F32, name=f"z_{m}")
        nc.scalar.activation(zt[:], zp[:], AFT.Copy, scale=1.0 / W1E_SCALE)
        z_sb.append(zt)

    # per-invocation accumulators for the deferred finalize
    zvec = fpool.tile([1, BL], F32, name="zvec")
    ctxall = fpool.tile([P, BL * KC], F32, name="ctxall")

    # --- pipelined main loop ---
    state = {}    # per-batch: z_parts tile + list of per-chunk ctx tiles
    pending = None  # chunk awaiting exp/broadcast/ctx emission
    prefetched = {}  # (b, ss) -> (et8, etb) issued a batch early

    def dma_chunk(b, ss):
        et8 = epool8.tile([P, KC * S_SUB], WT8, name="et8", tag="et8")
        nc.sync.dma_start(et8[:], encT8[b, ss])
        etb = epoolb.tile([P, KC * S_SUB], DT, name="etb", tag="etb")
        nc.sync.dma_start(etb[:], encTb[b, ss])
        return et8, etb

    def emit_post(pend):
        """exp, p-broadcast, and DVE context work for a finished chunk."""
        pb, pss, e_ps, etb_big = pend
        st = state[pb]
        p_row = spool.tile([1, S_SUB], DT, name="p_row", tag="p_row")
        nc.scalar.activation(p_row[:], e_ps[0:1, :], AFT.Exp,
                             accum_out=st["z_parts"][0:1, pss:pss + 1])
        # p broadcast on GpSimd (idle engine): frees the PE matmul + ACT
        # copy + a PSUM bank; bf16 output keeps the DVE in 4x perf mode
        pbc_sb = pbpool.tile([P, S_SUB], DT, name="pbc_sb", tag="pbc_sb")
        nc.gpsimd.partition_broadcast(pbc_sb[:], p_row[:])
        scratch = spool2.tile([P, KC * S_SUB], DT, name="scr", tag="scr")
        p_rep = pbc_sb[:].rearrange("p (o s) -> p o s",
                                    o=1).broadcast_to((P, KC, S_SUB))
        nc.vector.tensor_tensor(
            scratch[:].rearrange("p (k s) -> p k s", k=KC),
            etb_big[:].rearrange("p (k s) -> p k s", k=KC),
            p_rep, ALU.mult)
        cred = cpool.tile([P, KC], DT, name="cred", tag="cred")
        with nc.allow_low_precision(reason="bf16 cred keeps the DVE reduce "
                                    "in 4x mode; ~0.4% on a 2e-2 budget"):
            nc.vector.tensor_reduce(
                cred[:], scratch[:].rearrange("p (k s) -> p k s", k=KC),
                axis=mybir.AxisListType.X, op=ALU.add)
        st["creds"].append(cred)
        if pss == NSS - 1:
            emit_batch_reduce(pb)

    def emit_batch_reduce(pb):
        """Per-batch: Z partial sum + unnormalized ctx into the rep-wide
        accumulators. Runs overlapped with the next batch's matmuls."""
        st = state.pop(pb)
        nc.vector.tensor_reduce(zvec[0:1, pb:pb + 1], st["z_parts"][:],
                                axis=mybir.AxisListType.X, op=ALU.add)
        creds = st["creds"]
        dst = ctxall[:, pb * KC:(pb + 1) * KC]
        nc.vector.tensor_tensor(dst, creds[0][:], creds[1][:], ALU.add)
        nc.vector.tensor_tensor(dst, dst, creds[2][:], ALU.add)
        nc.vector.tensor_tensor(dst, dst, creds[3][:], ALU.add)

    for b in range(BL):
        state[b] = {
            "z_parts": spool.tile([1, NSS], F32, name="z_parts",
                                  tag="z_parts"),
            "creds": [],
        }
        # all four seq chunks of this batch resident; chunks are
        # prefetched into the pools during the PREVIOUS batch's m-loop so
        # the DMA burst doesn't align with the first matmuls that need it
        ets8, etsb = [], []
        for ss in range(NSS):
            if b == 0 and ss == 0:
                ets8.append(et8_0)
                etsb.append(etb_0)
                continue
            if (b, ss) in prefetched:
                e8, eb = prefetched.pop((b, ss))
                ets8.append(e8)
                etsb.append(eb)
                continue
            et8 = epool8.tile([P, KC * S_SUB], WT8, name="et8", tag="et8")
            nc.sync.dma_start(et8[:], encT8[b, ss])
            ets8.append(et8)
            etb = epoolb.tile([P, KC * S_SUB], DT, name="etb", tag="etb")
            nc.sync.dma_start(etb[:], encTb[b, ss])
            etsb.append(etb)
        eb0 = ppe.tile([P, S_SUB], F32, name="eb", tag="eb")
        eb1 = ppe.tile([P, S_SUB], F32, name="eb", tag="eb")
        e_slices = [eb0[0:2, :], eb0[32:34, :], eb1[0:2, :], eb1[32:34, :]]
        h_all = []
        for m in range(KC):
            hps = [ppa.tile([P, S_SUB], F32, name="hp", tag="ppa_t")
                   for _ in range(NSS)]
            h_cur = []
            for ss in range(NSS):
                for pl in range(NPL):
                    kp = pl if pl < NKP else pl - NKP
                    rhs = ets8[ss][:, kp * 2 * S_SUB:(kp + 1) * 2 * S_SUB
                                   ].rearrange("p (two s) -> p two s", two=2)
                    nc.tensor.matmul(
                        hps[ss][:], lhsT=w1e_pl(m, pl), rhs=rhs,
                        start=(pl == 0), stop=(pl == NPL - 1),
                        perf_mode=mybir.MatmulPerfMode.DoubleRow)
                h_sb = hpool.tile([P, S_SUB], DT, name="h_sb", tag="h_sb")
                nc.scalar.activation(h_sb[:], hps[ss][:], AFT.Tanh,
                                     bias=z_sb[m][:, b:b + 1],
                                     scale=1.0 / (W1E_SCALE * ENC_SCALE))
                h_cur.append(h_sb)
            h_all.append(h_cur)
            if b + 1 < BL and 1 <= m <= NSS:
                prefetched[(b + 1, m - 1)] = dma_chunk(b + 1, m - 1)
            if m == 0 and pending is not None:
                for pend in pending:
                    emit_post(pend)
                pending = None
        # e-scores: one contiguous bf16 matmul block per batch
        for m in range(KC):
            for ss in range(NSS):
                nc.tensor.matmul(e_slices[ss], lhsT=w2_t[m],
                                 rhs=h_all[m][ss][:],
                                 start=(m == 0), stop=(m == KC - 1))
        pending = [(b, ss, e_slices[ss], etsb[ss]) for ss in range(NSS)]
    for pend in pending:
        emit_post(pend)

    # --- deferred finalize: ctx / Z, one contiguous store ---
    zbc = spool.tile([P, BL], F32, name="zbc", tag="zbc")
    nc.gpsimd.partition_broadcast(zbc[:], zvec[:])
    lnz = spool.tile([P, BL], F32, name="lnz", tag="lnz")
    nc.scalar.activation(lnz[:], zbc[:], AFT.Ln)
    zr = spool.tile([P, BL], F32, name="zr", tag="zr")
    nc.scalar.activation(zr[:], lnz[:], AFT.Exp, scale=-1.0)
    ctxout = fpool.tile([P, BL * KC], F32, name="ctxout")
    for b in range(BL):
        nc.vector.tensor_scalar_mul(ctxout[:, b * KC:(b + 1) * KC],
                                    ctxall[:, b * KC:(b + 1) * KC],
                                    zr[:, b:b + 1])
    nc.sync.dma_start(out[:, :], ctxout[:])


def declare_inputs(nc, kind="ExternalInput"):
    encT8 = nc.dram_tensor("encT8", [BL, NSS, P, KC * S_SUB], WT8,
                           kind=kind).ap()
    encTb = nc.dram_tensor("encTb", [BL, NSS, P, KC * S_SUB], DT,
                           kind=kind).ap()
    w1e8 = nc.dram_tensor("w1e8", [P, KC * NPL * 2 * P], WT8,
                          kind=kind).ap()
    hidT = nc.dram_tensor("hidT", [D, BL], DT, kind=kind).ap()
    w1h = nc.dram_tensor("w1h", [D, D], WT, kind=kind).ap()
    b1r = nc.dram_tensor("b1r", [1, D], DT, kind=kind).ap()
    onesb = nc.dram_tensor("onesb", [1, BL], DT, kind=kind).ap()
    w2 = nc.dram_tensor("w2", [D, 2], DT, kind=kind).ap()
    onesr = nc.dram_tensor("onesr", [1, P], WT, kind=kind).ap()
    return encT8, encTb, w1e8, hidT, w1h, b1r, onesb, w2, onesr


def build_program():
    nc = bacc.Bacc("TRN2", target_bir_lowering=False, debug=False,
                   num_devices=NCORES)
    ins = declare_inputs(nc, kind="ExternalInput")
    # ctx in [partition, batch*KC] layout; host permutes back to [BL, D]
    out = nc.dram_tensor("ctx_out", [P, BL * KC], F32,
                         kind="ExternalOutput").ap()
    with tile.TileContext(nc) as tc:
        with ExitStack() as ctx:
            _body(ctx, tc, *ins, out)
    nc.compile()
    return nc


def prep_in_maps(inputs):
    import ml_dtypes
    bf16 = ml_dtypes.bfloat16
    fp8 = ml_dtypes.float8_e3m4
    e4m3 = ml_dtypes.float8_e4m3
    enc = np.asarray(inputs["encoder_outputs"], dtype=np.float32)
    hid = np.asarray(inputs["hidden_state"], dtype=np.float32)
    W1 = np.asarray(inputs["W1"], dtype=np.float32)
    b1 = np.asarray(inputs["b1"], dtype=np.float32)
    w2 = np.asarray(inputs["w2"], dtype=np.float32)

    # enc in [b, ss, p, k*512+s] tile layout, both e4m3 (x8) and bf16
    encr = enc.reshape(B, NSS, S_SUB, KC, P).transpose(0, 1, 4, 3, 2)
    encr = np.ascontiguousarray(encr).reshape(B, NSS, P, KC * S_SUB)
    enc8 = np.clip(encr * ENC_SCALE, -448, 448).astype(e4m3)
    encb = encr.astype(bf16)

    # W1_enc DoubleRow planes: [p, m-major blocks of (NPL, two, q)]
    W128 = np.clip(W1[:D] * W1E_SCALE, -448, 448)
    hi8 = W128.astype(e4m3)
    planes = np.empty((P, KC, NPL, 2, P), dtype=e4m3)
    hi_r = np.asarray(hi8).reshape(NKP, 2, P, KC, P).transpose(2, 3, 0, 1, 4)
    planes[:, :, :NKP] = hi_r
    if NPL > NKP:
        lo8 = (W128 - hi8.astype(np.float32)).astype(e4m3)
        lo_r = np.asarray(lo8).reshape(NKP, 2, P, KC, P
                                       ).transpose(2, 3, 0, 1, 4)
        planes[:, :, NKP:NPL] = lo_r[:, :, :NPL - NKP]
    w1e8 = planes.reshape(P, KC * NPL * 2 * P)

    w1h = np.clip(np.ascontiguousarray(W1[D:]) * W1E_SCALE,
                  -15.5, 15.5).astype(fp8)
    b1r = (b1 * W1E_SCALE).astype(bf16).reshape(1, D)
    onesb = np.ones((1, BL), dtype=bf16)
    w2c = np.zeros((D, 2), dtype=bf16)
    w2c[:, 0] = w2.astype(bf16)
    onesr_np = np.ones((1, P), dtype=fp8)
    in_maps = []
    for c in range(NCORES):
        sl = slice(c * BL, (c + 1) * BL)
        in_maps.append({
            "encT8": enc8[sl],
            "encTb": encb[sl],
            "w1e8": w1e8,
            "hidT": np.ascontiguousarray(hid[sl].T).astype(bf16),
            "w1h": w1h,
            "b1r": b1r,
            "onesb": onesb,
            "w2": w2c,
            "onesr": onesr_np,
        })
    return in_maps


_NC_CACHE = None


def unpack_out(arr):
    """Device layout [P, BL*KC] -> [BL, D]: [p, b*KC+k] = ctx[b, k*P+p]."""
    return (np.asarray(arr, dtype=np.float32).reshape(P, BL, KC)
            .transpose(1, 2, 0).reshape(BL, D))


def kernel(**inputs):
    global _NC_CACHE
    if _NC_CACHE is None:
        _NC_CACHE = build_program()
    nc = _NC_CACHE
    in_maps = prep_in_maps(inputs)
    res = run_bass_kernel_spmd(nc, in_maps, core_ids=list(range(NCORES)))
    out = np.empty((B, D), dtype=np.float32)
    for c in range(NCORES):
        out[c * BL:(c + 1) * BL] = unpack_out(res.results[c]["ctx_out"])
    return out
